# revision 1
# baseline (speedup 1.0000x reference)
"""AgentAwareAttention Trainium2 kernel.

Full (unsharded) inputs -> full output.  Internally: 16 (batch, head) pairs
sharded 2-per-core across 8 NeuronCores; host pre-transposes operands and
sorts the sequence by agent id so the agent-equality mask becomes
block-diagonal.

Device-side structure (per core, heads h0/h1 of one batch):
  - projections:  QT = [q|qs] (bf16), QM = [-q|qs], KT = [k|ks]
  - main scores:  KT[k].T @ QT[q]           (32-contract, bf16)
  - delta scores: mask(KT)[k;ks].T @ QM[-q;qs]  adds (qs.ks - q.k) on the
    block-diagonal rects; masks are zero-padded KT column tiles built on the
    idle Pool engine (zeroed via KT*0 so the zeroing DEPENDS on KT and the
    scheduler cannot hoist every mask ahead of the first dep-ready copy).
  - softmax exp is split across TWO engines: true Exp on Act, and a
    Schraudolph bit-trick exp on DVE (fp8 bits = round(x*8/ln2 + 55.55)
    written through an int8 bitcast) -- the softmax normalization cancels
    the correlated approximation error (measured end-to-end ~6e-3).
  - pslab + v_sb are fp8e4m3; attention-weights@V runs as fp8 DoubleRow
    matmuls ([128, 2, f] operand views, 0.5 cycles/row; v_sb tile blocks
    padded to 80 so the dual-fp8 Ldweights outer step is 16B-aligned).
  - PV uses ones-augmented V (rowsum for free); normalize + merged-head
    64-contract out-proj; j-pairs share one psum bank / osb / output DMA.
  - each unit's last score group runs from the self psum pool so the next
    unit's first group reuses a score buffer freed two groups earlier
    (no exp-stream stall at unit boundaries).

All four projection chunks are emitted BEFORE any score unit that reads
them (every S(u) reads all 16 KT key tiles -- interleaving proj(lc) behind
S(u) is a read-before-write race that CoreSim catches).  Emission is
software-pipelined over units u=(lc,h): S(u)=scores+exp, P(u)=PV,
OA/OB(lc)=normalize+outproj halves, ordered so no engine's in-order
queue blocks another engine's stream.

Shapes (hardcoded): L=2048, N=2, E=256, H=8, D=32, N_AGENTS=16.
"""

import os

import numpy as np
from ml_dtypes import bfloat16


L = 2048
NB = 2        # batch
E = 256       # embed dim
H = 8         # heads
D = 32        # head dim
NCORES = 8
LC = 512      # l-chunk (moving-operand free dim)
NT = L // 128   # 16 s'-tiles of 128
NLC = L // LC   # 4 l-chunks

_PROGRAM_CACHE = {}


def _block_structure(ids):
    """Sort positions by agent id.  Returns perm and per-agent ranges in
    permuted space."""
    ids = np.asarray(ids)
    perm = np.argsort(ids, kind="stable")
    sids = ids[perm]
    bounds = [0]
    for i in range(1, len(sids)):
        if sids[i] != sids[i - 1]:
            bounds.append(i)
    bounds.append(len(sids))
    blocks = [(bounds[i], bounds[i + 1]) for i in range(len(bounds) - 1)]
    return perm, blocks


def _rects(blocks):
    """rects[(t, lc)] -> list of (r0, r1, c0, c1): the part of diagonal block
    (rows x cols, both = the block's range) that intersects s'-tile t
    (rows [128t,128t+128)) and l-chunk lc (cols [LC*lc, LC*lc+LC)), in
    tile-local coordinates."""
    rects = {}
    for (b0, b1) in blocks:
        for t in range(NT):
            r0 = max(b0, 128 * t)
            r1 = min(b1, 128 * t + 128)
            if r0 >= r1:
                continue
            for lc in range(NLC):
                c0 = max(b0, LC * lc)
                c1 = min(b1, LC * lc + LC)
                if c0 >= c1:
                    continue
                rects.setdefault((t, lc), []).append(
                    (r0 - 128 * t, r1 - 128 * t, c0 - LC * lc, c1 - LC * lc)
                )
    return rects


def _build_program(rects):
    import numpy as np
    import concourse.mybir as mybir
    import concourse.tile as tile
    from concourse import bacc

    f32 = mybir.dt.float32
    bf16 = mybir.dt.bfloat16
    fp8 = mybir.dt.float8e4
    add = mybir.AluOpType.add
    mult = mybir.AluOpType.mult

    # masks needed: (t, r0, r1) -> arena index; full-tile rects use KT directly
    mask_idx = {}
    for (t, lc), rl in sorted(rects.items()):
        for (r0, r1, c0, c1) in rl:
            if (r0, r1) == (0, 128):
                continue
            key = (t, r0, r1)
            if key not in mask_idx:
                mask_idx[key] = len(mask_idx)
    n_masks = max(1, len(mask_idx))

    nc = bacc.Bacc(None)

    xT_d = nc.declare_dram_parameter("xT", [E, L], bf16, isOutput=False)
    # wpack: [wqkv half0 (320) | wqkv half1 (320) | wo (256) | xt0 lc0 (512)
    #         | xt1 lc0 (512)] -- lc0's xt rides the same DMA as the weights
    wpack_d = nc.declare_dram_parameter("wpack", [128, 1920], bf16, isOutput=False)
    bias4_d = nc.declare_dram_parameter("bias4", [128, 4], f32, isOutput=False)
    # bf16 output halves the tail's serial out-DMA transfers; host unshard
    # accumulates in fp32 (partials are small -> negligible rounding)
    out_d = nc.declare_dram_parameter("out", [L, E], bf16, isOutput=True)

    with tile.TileContext(nc) as tc:
        with (
            tc.tile_pool(name="consts", bufs=1) as consts,
            tc.tile_pool(name="pslab", bufs=5) as pslab_pool,
            tc.tile_pool(name="onorm", bufs=6) as onorm_pool,
            tc.tile_pool(name="small", bufs=8) as small_pool,
            tc.tile_pool(name="outsb", bufs=6) as outsb_pool,
            tc.tile_pool(name="ps_score", bufs=2, space="PSUM") as ps_score,
            tc.tile_pool(name="ps_self", bufs=2, space="PSUM") as ps_self,
            tc.tile_pool(name="ps_oacc", bufs=2, space="PSUM") as ps_oacc,
        ):
            # ---- constant loads -------------------------------------------
            # HWDGE serializes DMA issue at ~625ns each, so everything
            # constant rides ONE packed DMA; xt is chunked so proj(0) can
            # start as soon as its first 512 columns land.
            xt = [consts.tile([128, L], bf16, tag=f"xt{i}", name=f"xt{i}")
                  for i in range(2)]
            wpack = consts.tile([128, 1920], bf16, tag="wpack", name="wpack")
            bias4 = consts.tile([128, 4], f32, tag="bias4", name="bias4")
            wq = [wpack[:, 320 * i:320 * i + 128] for i in range(2)]
            wk = [wpack[:, 320 * i + 128:320 * i + 256] for i in range(2)]
            wv = [wpack[:, 320 * i + 256:320 * i + 320] for i in range(2)]
            bq_t = bias4[:, 0:1]
            bk_t = bias4[:, 1:2]
            bqm_t = bias4[:, 2:3]
            sg_t = bias4[:, 3:4]
            wo_pair = wpack[0:64, 640:896]
            xt_lc0 = [wpack[:, 896 + 512 * i:896 + 512 * (i + 1)]
                      for i in range(2)]

            def xs(i, a, b):
                """xt[i] columns [a,b) -- lc0 columns live inside wpack."""
                if b <= LC:
                    return xt_lc0[i][:, a:b]
                return xt[i][:, a:b]

            nc.sync.dma_start(wpack, wpack_d[:, :])
            nc.sync.dma_start(bias4, bias4_d[:, :])
            for i in range(2):
                nc.sync.dma_start(xt[i][:, LC:],
                                  xT_d[128 * i:128 * (i + 1), LC:])

            QT = consts.tile([128, L], bf16, tag="QT", name="QT")
            QM = consts.tile([128, L], bf16, tag="QM", name="QM")
            KT = consts.tile([128, L], bf16, tag="KT", name="KT")
            marena = consts.tile([128, n_masks, 128], bf16, tag="marena",
                                 name="marena")
            # per-tile block padded to 80 (16B-aligned stride: the ISA's
            # dual-row fp8 Ldweights requires outer step % 16 == 0)
            v_sb = consts.tile([128, NT, 80], fp8, tag="vsb", name="v_sb")

            # PE warm-up: TRN2 pstate ramps (0.65 -> 1.2 -> 2.4 GHz) key off
            # continuous PE activity; a few throwaway matmuls on a memset
            # tile keep the projection matmuls off the cold clock.
            dummy = consts.tile([128, 512], bf16, tag="dummy", name="dummy")
            dume = consts.tile([1, 1], bf16, tag="dume", name="dume")
            ones1 = consts.tile([1, 32], f32, tag="ones1", name="ones1")
            nc.gpsimd.memset(ones1, 1.0)
            nc.gpsimd.memset(dummy, 0.0)
            nc.gpsimd.memset(dume, 0.0)
            nc.gpsimd.memset(v_sb, 1.0)
            # hoist the Exp act-table load off the critical path
            nc.scalar.activation(dume, dummy[0:1, 0:1],
                                 mybir.ActivationFunctionType.Exp)
            for w in range(2):
                pdum = ps_self.tile([128, 512], f32, tag="self",
                                    name="ps_self_t")
                nc.tensor.matmul(pdum, dummy[:, 0:128], dummy,
                                 start=True, stop=True)

            def delta_lhs(t, r0, r1):
                if (r0, r1) == (0, 128):
                    return KT[:, 128 * t:128 * (t + 1)]
                m = mask_idx[(t, r0, r1)]
                return marena[:, m, :]

            def proj(lc):
                sl = slice(LC * lc, LC * (lc + 1))
                pk = ps_self.tile([128, 512], f32, tag="self", name="ps_self_t")
                nc.tensor.matmul(pk, wk[0], xs(0, sl.start, sl.stop),
                                 start=True, stop=False)
                nc.tensor.matmul(pk, wk[1], xs(1, sl.start, sl.stop),
                                 start=False, stop=True)
                nc.vector.tensor_scalar(
                    out=KT[:, sl], in0=pk, scalar1=bk_t, scalar2=None, op0=add)
                # mask tiles sourced from this KT chunk (idle Pool engine).
                # zeroing via KT*0 (not memset) so it DEPENDS on this KT
                # chunk -- otherwise the scheduler hoists every no-dep memset
                # ahead of the first dep-ready copy and starves lc0's masks.
                for (t, r0, r1), m in sorted(mask_idx.items(), key=lambda kv: kv[1]):
                    if t // 4 == lc:
                        nc.gpsimd.tensor_scalar(
                            out=marena[:, m, :],
                            in0=KT[:, 128 * t:128 * (t + 1)],
                            scalar1=0.0, scalar2=None, op0=mult)
                        nc.gpsimd.tensor_copy(
                            marena[:, m, r0:r1],
                            KT[:, 128 * t + r0:128 * t + r1])
                pq = ps_self.tile([128, 512], f32, tag="self", name="ps_self_t")
                nc.tensor.matmul(pq, wq[0], xs(0, sl.start, sl.stop),
                                 start=True, stop=False)
                nc.tensor.matmul(pq, wq[1], xs(1, sl.start, sl.stop),
                                 start=False, stop=True)
                nc.vector.tensor_scalar(
                    out=QT[:, sl], in0=pq, scalar1=bq_t, scalar2=None, op0=add)
                if lc == 0:
                    # Act idles pre-exp; DVE's scheduler otherwise orders KT1
                    # ahead of QM0 and delays the first delta matmuls
                    nc.scalar.activation(
                        QM[:, sl], pq, mybir.ActivationFunctionType.Identity,
                        bias=bqm_t, scale=sg_t)
                else:
                    nc.vector.tensor_scalar(
                        out=QM[:, sl], in0=pq, scalar1=sg_t, scalar2=bqm_t,
                        op0=mult, op1=add)

            def vbuild():
                for t in range(NT):
                    ts = slice(128 * t, 128 * (t + 1))
                    pv = ps_self.tile([128, 512], f32, tag="self",
                                      name="ps_self_t")
                    nc.tensor.matmul(pv[:, 0:64], xs(0, ts.start, ts.stop),
                                     wv[0], start=True, stop=False)
                    nc.tensor.matmul(pv[:, 0:64], xs(1, ts.start, ts.stop),
                                     wv[1], start=False, stop=True)
                    nc.vector.tensor_copy(v_sb[:, t, 0:32], pv[:, 0:32])
                    nc.vector.tensor_copy(v_sb[:, t, 33:65], pv[:, 32:64])

            pslabs = {}
            oaccs = {}
            ons = {}

            # score groups: contiguous t-runs of 3 (then the leftover t15)
            # -> 6 exps of width <=1536 per (lc,h) instead of 8x1024,
            # amortizing the Act engine's fixed per-instruction access cost
            SGROUPS = [[0, 1], [2, 3], [4, 5], [6, 7], [8, 9], [10, 11],
                       [12, 13], [14, 15]]
            # unit 0 leads with a narrow [t0] group: exp0 fires right after
            # QM0 lands instead of waiting for a full 1024-wide group
            SGROUPS0 = [[0], [1], [2, 3], [4, 5], [6, 7], [8, 9], [10, 11],
                        [12, 13], [14, 15]]
            # units 1-6: two groups each on DVE; final unit alternates so
            # Act and DVE drain unit 7's exps in parallel (shorter tail)
            DVE_EXP_GROUPS = {0, 2, 4}
            U0_DVE = 1
            NSELF = 2
            DVE_EXP_GROUPS_ODD = {0, 2, 4}
            DVE_EXP_GROUPS_LAST = {1, 3, 5}

            def S(u, g0=0, g1=None):
                lc, h = u // 2, u % 2
                groups = SGROUPS0 if u == 0 else SGROUPS
                if g1 is None:
                    g1 = len(groups)
                lsl = slice(LC * lc, LC * (lc + 1))
                qb = 64 * h
                if g0 == 0:
                    pslab = pslab_pool.tile([128, NT * 512], fp8, tag="pslab",
                                            name="pslab")
                    pslabs[u] = pslab
                else:
                    pslab = pslabs[u]
                def score_tiles_into(ps2, ts):
                    for k, t in enumerate(ts):
                        tsl = slice(128 * t, 128 * (t + 1))
                        o = 512 * k
                        rl = rects.get((t, lc), [])
                        nc.tensor.matmul(
                            ps2[:, o:o + 512],
                            KT[qb:qb + 32, tsl], QT[qb:qb + 32, lsl],
                            start=True, stop=(not rl), tile_position=(qb, 0))
                        for i, (r0, r1, c0, c1) in enumerate(rl):
                            mk = delta_lhs(t, r0, r1)
                            nc.tensor.matmul(
                                ps2[:, o + c0:o + c1],
                                mk[qb:qb + 64, :],
                                QM[qb:qb + 64, LC * lc + c0:LC * lc + c1],
                                start=False, stop=(i == len(rl) - 1),
                                tile_position=(qb, 0))

                for g in range(g0, g1):
                    ts = groups[g]
                    w = 512 * len(ts)
                    if g >= len(groups) - NSELF and len(ts) == 2:
                        # last group of each unit runs from the (mostly idle)
                        # self pool as two 512-wide tiles: the NEXT unit's
                        # first group then reuses a score buffer freed two
                        # groups earlier -- kills the unit-boundary exp stall
                        for t in ts:
                            pa = ps_self.tile([128, 512], f32, tag="self",
                                              name="ps_self_t")
                            score_tiles_into(pa, [t])
                            nc.scalar.activation(
                                pslab[:, 512 * t:512 * (t + 1)], pa,
                                mybir.ActivationFunctionType.Exp)
                        continue
                    ps2 = ps_score.tile([128, 1024], f32, tag="score",
                                        name="ps2")
                    score_tiles_into(ps2, ts)
                    dst = pslab[:, 512 * ts[0]:512 * ts[0] + w]
                    if u == NLC * 2 - 1:
                        dve_g = DVE_EXP_GROUPS_LAST
                    elif u % 2 == 1:
                        dve_g = DVE_EXP_GROUPS_ODD
                    else:
                        dve_g = DVE_EXP_GROUPS
                    if g in dve_g and u >= U0_DVE:
                        # Schraudolph bit-trick exp on the otherwise-idle DVE:
                        # bf16 bits = round(x*128/ln2 + (16256-C)); softmax
                        # normalization cancels the correlated approx error
                        # (measured end-to-end ~2e-3 even at 100% offload)
                        nc.vector.tensor_scalar(
                            out=dst.bitcast(mybir.dt.int8), in0=ps2[:, 0:w],
                            scalar1=float(8.0 / np.log(2.0)),
                            scalar2=56.0 - 0.45,
                            op0=mult, op1=add)
                    else:
                        nc.scalar.activation(
                            dst, ps2[:, 0:w],
                            mybir.ActivationFunctionType.Exp)

            def P(u):
                # fp8 DoubleRow: each matmul contracts TWO 128-row key tiles
                # ([128, 2, f] operand views) at 0.5 cycles/row -- PV cost
                # drops 4x vs bf16 single-tile matmuls
                lc, h = u // 2, u % 2
                pslab = pslabs[u]
                oacc = ps_oacc.tile([33, 512], f32, tag="oacc", name="oacc")
                oaccs[u] = oacc
                for gp in range(NT // 2):
                    rhs = pslab[:, 1024 * gp:1024 * (gp + 1)].rearrange(
                        "p (two f) -> p two f", two=2)
                    nc.tensor.matmul(
                        oacc, v_sb[:, 2 * gp:2 * gp + 2, 33 * h:33 * h + 33],
                        rhs,
                        start=(gp == 0), stop=(gp == NT // 2 - 1),
                        perf_mode=mybir.MatmulPerfMode.DoubleRow)

            def OA(lc):
                # h0's normalize emitted early (right after P(2lc)): halves
                # the serial DVE clump that otherwise delays DVE's next exp
                # group and stalls the Act stream via the psum recycle
                on = onorm_pool.tile([64, 512], bf16, tag="onorm", name="on")
                ons[lc] = on
                oacc = oaccs[2 * lc]
                rr = small_pool.tile([1, 512], f32, tag="rr", name="rr")
                nc.vector.reciprocal(rr, oacc[32:33, :])
                rb = small_pool.tile([32, 512], f32, tag="rb", name="rb")
                nc.gpsimd.partition_broadcast(rb, rr)
                nc.vector.tensor_mul(on[0:32, :], oacc[0:32, :], rb)

            def OB(lc):
                on = ons[lc]
                oacc = oaccs[2 * lc + 1]
                rr = small_pool.tile([1, 512], f32, tag="rr", name="rr")
                nc.vector.reciprocal(rr, oacc[32:33, :])
                rb = small_pool.tile([32, 512], f32, tag="rb", name="rb")
                nc.gpsimd.partition_broadcast(rb, rr)
                nc.vector.tensor_mul(on[32:64, :], oacc[0:32, :], rb)
                act_osb = lc == NLC - 1
                for jj in range(2):
                    po = ps_self.tile([128, 512], f32, tag="self",
                                      name="ps_self_t")
                    for j2 in range(2):
                        j = 2 * jj + j2
                        onj = on[:, 128 * j:128 * (j + 1)]
                        nc.tensor.matmul(po[:, 256 * j2:256 * (j2 + 1)],
                                         onj, wo_pair, start=True, stop=True)
                    osb = outsb_pool.tile([128, 512], bf16, tag="outsb",
                                          name="osb")
                    if act_osb and jj == 0:
                        nc.scalar.activation(osb, po,
                                             mybir.ActivationFunctionType.Copy)
                    else:
                        nc.vector.tensor_copy(osb, po)
                    r0 = LC * lc + 256 * jj
                    dst = out_d[r0:r0 + 256, :].rearrange(
                        "(j r) c -> r j c", j=2)
                    nc.sync.dma_start(dst, osb.rearrange("r (j c) -> r j c", j=2))

            def O(lc):
                # normalize both heads into one [64, 512] tile, then a single
                # 64-contract out-proj matmul per j-tile; j-pairs share one
                # psum bank / osb / output DMA.  The last chunk's copies ride
                # the (by then idle) Act engine to shorten the tail.
                on = onorm_pool.tile([64, 512], bf16, tag="onorm", name="on")
                split = False  # jj-half pipelining measured slower
                act_osb = lc == NLC - 1

                def norm(h, csl):
                    oacc = oaccs[2 * lc + h]
                    rr = small_pool.tile([1, 512], f32, tag="rr", name="rr")
                    rrv = rr[:, csl]
                    nc.vector.reciprocal(rrv, oacc[32:33, csl])
                    rb = small_pool.tile([32, 512], f32, tag="rb", name="rb")
                    rbv = rb[:, csl]
                    nc.gpsimd.partition_broadcast(rbv, rrv)
                    nc.vector.tensor_mul(on[32 * h:32 * (h + 1), csl],
                                         oacc[0:32, csl], rbv)

                if not split:
                    for h in range(2):
                        norm(h, slice(0, 512))
                for jj in range(2):
                    if split:
                        for h in range(2):
                            norm(h, slice(256 * jj, 256 * (jj + 1)))
                    po = ps_self.tile([128, 512], f32, tag="self",
                                      name="ps_self_t")
                    for j2 in range(2):
                        j = 2 * jj + j2
                        onj = on[:, 128 * j:128 * (j + 1)]
                        nc.tensor.matmul(po[:, 256 * j2:256 * (j2 + 1)],
                                         onj, wo_pair, start=True, stop=True)
                    osb = outsb_pool.tile([128, 512], bf16, tag="outsb",
                                          name="osb")
                    if act_osb and jj == 0:
                        nc.scalar.activation(osb, po,
                                             mybir.ActivationFunctionType.Copy)
                    else:
                        nc.vector.tensor_copy(osb, po)
                    r0 = LC * lc + 256 * jj
                    dst = out_d[r0:r0 + 256, :].rearrange(
                        "(j r) c -> r j c", j=2)
                    nc.sync.dma_start(dst, osb.rearrange("r (j c) -> r j c", j=2))

            # ---- software-pipelined emission ------------------------------
            # S(0)'s tp group k reads key tiles t=2k..2k+3 from KT chunk
            # lc=k//2, so interleaving with proj(lc) is program-order safe
            proj(0)
            S(0, 0, 3)
            proj(1)
            S(0, 3, 5)
            proj(2)
            S(0, 5, 7)
            proj(3)
            S(0, 7, 9)
            S(1)
            vbuild()
            P(0)
            S(2)
            P(1)
            OA(0)
            S(3)
            OB(0)
            P(2)
            S(4)
            P(3)
            OA(1)
            S(5)
            OB(1)
            P(4)
            S(6)
            P(5)
            OA(2)
            S(7)
            OB(2)
            P(6)
            P(7)
            O(3)
    nc.finalize()
    return nc


def _prep_inputs(query, in_proj_weight, in_proj_bias, in_proj_weight_self,
                 in_proj_bias_self, out_proj_weight, perm):
    """Per-core input maps (host-side transposes, permutation, scaling)."""
    scaling = np.float32(D ** -0.5)
    q_perm = np.asarray(query)[perm]          # (L, NB, E)

    Wq = np.asarray(in_proj_weight[0:E])
    Wk = np.asarray(in_proj_weight[E:2 * E])
    Wv = np.asarray(in_proj_weight[2 * E:3 * E])
    Wqs = np.asarray(in_proj_weight_self[0:E])
    Wks = np.asarray(in_proj_weight_self[E:2 * E])
    bq = np.asarray(in_proj_bias[0:E])
    bk = np.asarray(in_proj_bias[E:2 * E])
    bqs = np.asarray(in_proj_bias_self[0:E])
    bks = np.asarray(in_proj_bias_self[E:2 * E])
    WoT = np.ascontiguousarray(np.asarray(out_proj_weight).T)  # (E, E)

    xTs = [np.ascontiguousarray(q_perm[:, n, :].T).astype(bfloat16)
           for n in range(NB)]

    sgn = np.concatenate([-np.ones(32), np.ones(32),
                          -np.ones(32), np.ones(32)]).astype(np.float32)

    in_maps = []
    for c in range(NCORES):
        n = c // 4
        h0 = (2 * c) % H
        h1 = h0 + 1

        def hsl(W, h):
            return W[D * h:D * (h + 1)]

        wq_c = np.concatenate(
            [hsl(Wq, h0), hsl(Wqs, h0), hsl(Wq, h1), hsl(Wqs, h1)], 0) * scaling
        wk_c = np.concatenate(
            [hsl(Wk, h0), hsl(Wks, h0), hsl(Wk, h1), hsl(Wks, h1)], 0)
        wv_c = np.concatenate([hsl(Wv, h0), hsl(Wv, h1)], 0)
        bq_c = np.concatenate(
            [hsl(bq, h0), hsl(bqs, h0), hsl(bq, h1), hsl(bqs, h1)], 0) * scaling
        bk_c = np.concatenate(
            [hsl(bk, h0), hsl(bks, h0), hsl(bk, h1), hsl(bks, h1)], 0)
        wo_c = np.concatenate([WoT[D * h0:D * (h0 + 1)],
                               WoT[D * h1:D * (h1 + 1)]], 0)

        wqkv_c = np.concatenate([wq_c.T, wk_c.T, wv_c.T], axis=1)  # (E, 320)
        bias4_c = np.stack([bq_c, bk_c, sgn * bq_c, sgn], axis=1)  # (128, 4)
        wpack = np.zeros((128, 1920), dtype=np.float32)
        wpack[:, 0:320] = wqkv_c[0:128]
        wpack[:, 320:640] = wqkv_c[128:256]
        wpack[0:64, 640:896] = wo_c                                # (64, 256)
        wpack[:, 896:1408] = xTs[n][0:128, 0:512].astype(np.float32)
        wpack[:, 1408:1920] = xTs[n][128:256, 0:512].astype(np.float32)

        in_maps.append({
            "xT": xTs[n],
            "wpack": wpack.astype(bfloat16),
            "bias4": np.ascontiguousarray(bias4_c).astype(np.float32),
        })
    return in_maps


def _run(nc, in_maps, trace=False):
    from concourse.bass_utils import run_bass_kernel_spmd
    return run_bass_kernel_spmd(nc, in_maps, list(range(NCORES)), trace=trace)


def kernel(query, in_proj_weight, in_proj_bias, in_proj_weight_self,
           in_proj_bias_self, out_proj_weight, out_proj_bias,
           q_identities, k_identities, _trace=False, _return_br=False):
    ids = np.asarray(q_identities)
    perm, blocks = _block_structure(ids)

    key = ids.tobytes()
    if key not in _PROGRAM_CACHE:
        _PROGRAM_CACHE[key] = _build_program(_rects(blocks))
    nc = _PROGRAM_CACHE[key]

    in_maps = _prep_inputs(query, in_proj_weight, in_proj_bias,
                           in_proj_weight_self, in_proj_bias_self,
                           out_proj_weight, perm)
    br = _run(nc, in_maps, trace=_trace)

    # ---- unshard --------------------------------------------------------------
    # host bias: out_proj_bias + contribution of the v-bias through out_proj
    bias_total = (np.asarray(out_proj_bias)
                  + np.asarray(out_proj_weight) @ np.asarray(in_proj_bias)[2 * E:])
    out = np.zeros((L, NB, E), dtype=np.float32)
    for c in range(NCORES):
        n = c // 4
        out[:, n, :] += np.asarray(br.results[c]["out"], dtype=np.float32)
    out += bias_total[None, None, :].astype(np.float32)
    # un-permute rows
    out_full = np.empty_like(out)
    out_full[perm] = out
    if _return_br:
        return out_full, br
    return out_full



# revision 38
# speedup vs baseline: 1.0639x; 1.0639x over previous
"""AgentAwareAttention Trainium2 kernel (v3: fp8 DoubleRow score matmuls).

Full (unsharded) inputs -> full output.  16 (batch, head) pairs sharded
2-per-core across 8 NeuronCores; host pre-transposes operands and sorts the
sequence by agent id so the agent-equality mask becomes block-diagonal.

Device-side structure (per core, heads h0/h1 of one batch):
  - projections into psum; drains write fp8 staging kstg=[k|ks] and
    qstg=[-q|qs] per head-pair rows (sign + scaling folded into the Q
    drain's per-partition scalars); scores run straight off the staging.
  - main scores: fp8 matmul per (t, lc, h): stationary kstg rows
    [64h, 64h+32) (k), moving qstg same rows (-q) -> psum = -k.q
    (NEGATED scores; the exp applies scale=-1).
  - delta scores (block-diagonal rects): 64-contract fp8 matmuls with
    zero-padded sign-baked masks -[k~; k~s] (single Pool tensor_scalar
    per mask from kstg); psum on rect rows = -qs.k~s (self score).
  - softmax exp split Act (native Exp, scale=-1) / DVE (negated
    Schraudolph bit-trick into fp8); pslab fp8.
  - PV: fp8 DoubleRow, both heads into ONE [128, 512] psum tile
    (h0 stationary v_sb[.., 0:64] -> rows 0:64, h1 [.., 33:97] -> 64:128;
    ones-cols at 32/65 give rowsums at rows 32 and 96 for free).
  - normalize: two [1,512] reciprocals into rr [1,1024] f32r, PE rank-1
    sel-matmuls broadcast 1/s into rb psum rows {0:32, 64:96} (zeros in
    32:64), ONE tensor_mul over [0:96] -> on bf16 (junk rows x 0 = 0),
    out-proj contracts 96 rows against zero-middle wo96.
  - j-pairs share one psum bank / osb / output DMA (bf16 out).

Shapes (hardcoded): L=2048, N=2, E=256, H=8, D=32, N_AGENTS=16.
"""

import numpy as np
from ml_dtypes import bfloat16, float8_e4m3


L = 2048
NB = 2        # batch
E = 256       # embed dim
H = 8         # heads
D = 32        # head dim
NCORES = 8
LC = 512      # l-chunk (moving-operand free dim)
NT = L // 128   # 16 s'-tiles of 128
NLC = L // LC   # 4 l-chunks

_PROGRAM_CACHE = {}


def _block_structure(ids):
    """Sort positions by agent id.  Returns perm and per-agent ranges in
    permuted space."""
    ids = np.asarray(ids)
    perm = np.argsort(ids, kind="stable")
    sids = ids[perm]
    bounds = [0]
    for i in range(1, len(sids)):
        if sids[i] != sids[i - 1]:
            bounds.append(i)
    bounds.append(len(sids))
    blocks = [(bounds[i], bounds[i + 1]) for i in range(len(bounds) - 1)]
    return perm, blocks


def _rects(blocks):
    """rects[(t, lc)] -> list of (r0, r1, c0, c1): the part of diagonal block
    (rows x cols, both = the block's range) that intersects s'-tile t
    (rows [128t,128t+128)) and l-chunk lc (cols [LC*lc, LC*lc+LC)), in
    tile-local coordinates."""
    rects = {}
    for (b0, b1) in blocks:
        for t in range(NT):
            r0 = max(b0, 128 * t)
            r1 = min(b1, 128 * t + 128)
            if r0 >= r1:
                continue
            for lc in range(NLC):
                c0 = max(b0, LC * lc)
                c1 = min(b1, LC * lc + LC)
                if c0 >= c1:
                    continue
                rects.setdefault((t, lc), []).append(
                    (r0 - 128 * t, r1 - 128 * t, c0 - LC * lc, c1 - LC * lc)
                )
    return rects


def _build_program(rects):
    import numpy as np
    import concourse.mybir as mybir
    import concourse.tile as tile
    from concourse import bacc

    f32 = mybir.dt.float32
    f32r = mybir.dt.float32r
    bf16 = mybir.dt.bfloat16
    fp8 = mybir.dt.float8e4
    add = mybir.AluOpType.add
    mult = mybir.AluOpType.mult
    DR = mybir.MatmulPerfMode.DoubleRow

    # masks: lc0 rects use staging-layout [128, w] slabs (no fold wait on
    # the pipeline fill); lc>0 rects use DoubleRow-layout [64, 2, w] slabs
    # built from the folded QK pack
    mask0_idx = {}
    maskd_idx = {}
    for (t, lc), rl in sorted(rects.items()):
        for (r0, r1, c0, c1) in rl:
            key = (t, r0, r1)
            if lc == 0:
                if key not in mask0_idx:
                    mask0_idx[key] = len(mask0_idx)
            elif key not in maskd_idx:
                maskd_idx[key] = len(maskd_idx)
    n_masks0 = max(1, len(mask0_idx))
    n_masksd = max(1, len(maskd_idx))

    nc = bacc.Bacc(None)

    xT_d = nc.declare_dram_parameter("xT", [E, L], bf16, isOutput=False)
    # wpack: [wqkv half0 (320) | wqkv half1 (320) | wo96 (256) | xt0 lc0 (512)
    #         | xt1 lc0 (512)] -- lc0's xt rides the same DMA as the weights
    wpack_d = nc.declare_dram_parameter("wpack", [128, 1920], bf16, isOutput=False)
    bias4_d = nc.declare_dram_parameter("bias4", [128, 4], f32, isOutput=False)
    # f32r ones-selector (cols 64:96) for the tail's PE rank-1 broadcast
    aux_d = nc.declare_dram_parameter("aux", [1, 128], f32r, isOutput=False)
    # bf16 output halves the tail's serial out-DMA transfers; host unshard
    # accumulates in fp32 (partials are small -> negligible rounding)
    out_d = nc.declare_dram_parameter("out", [L, E], bf16, isOutput=True)

    with tile.TileContext(nc) as tc:
        with (
            tc.tile_pool(name="consts", bufs=1) as consts,
            tc.tile_pool(name="pslab", bufs=5) as pslab_pool,
            tc.tile_pool(name="onorm", bufs=6) as onorm_pool,
            tc.tile_pool(name="small", bufs=8) as small_pool,
            tc.tile_pool(name="outsb", bufs=6) as outsb_pool,
            tc.tile_pool(name="ps_score", bufs=2, space="PSUM") as ps_score,
            tc.tile_pool(name="ps_self", bufs=2, space="PSUM") as ps_self,
            tc.tile_pool(name="ps_oacc", bufs=2, space="PSUM") as ps_oacc,
        ):
            # ---- constant loads -------------------------------------------
            # HWDGE serializes DMA issue at ~625ns each, so the constants
            # ride ONE packed DMA; xt is chunked so proj(0) can start as
            # soon as its first 512 columns land.
            xt = [consts.tile([128, L], bf16, tag=f"xt{i}", name=f"xt{i}")
                  for i in range(2)]
            wpack = consts.tile([128, 1920], bf16, tag="wpack", name="wpack")
            bias4 = consts.tile([128, 4], f32, tag="bias4", name="bias4")
            aux = consts.tile([1, 128], f32r, tag="aux", name="aux")
            wq = [wpack[:, 320 * i:320 * i + 128] for i in range(2)]
            wk = [wpack[:, 320 * i + 128:320 * i + 256] for i in range(2)]
            wv = [wpack[:, 320 * i + 256:320 * i + 320] for i in range(2)]
            bk_t = bias4[:, 0:1]       # k bias (fold order)
            bqs_t = bias4[:, 1:2]      # sgn * scaled q bias
            sg_t = bias4[:, 2:3]       # sgn (-1 on q rows, +1 on qs rows)
            wo96 = wpack[0:96, 640:896]
            xt_lc0 = [wpack[:, 896 + 512 * i:896 + 512 * (i + 1)]
                      for i in range(2)]

            def xs(i, a, b):
                """xt[i] columns [a,b) -- lc0 columns live inside wpack."""
                if b <= LC:
                    return xt_lc0[i][:, a:b]
                return xt[i][:, a:b]

            # fp8 staging for drains, one per lc (S(0)/S(1) score directly
            # from lc0's staging; fold DMAs read all of them)
            kstg = [consts.tile([128, LC], fp8, tag=f"kstg{i}", name=f"kstg{i}")
                    for i in range(4)]
            qstg = [consts.tile([128, LC], fp8, tag=f"qstg{i}", name=f"qstg{i}")
                    for i in range(4)]
            marena2 = consts.tile([128, n_masks0, 128], fp8, tag="marena2",
                                  name="marena2")
            # DoubleRow operand pack for lc>0 scores: partitions 0:32 = h0
            # dims, 32:64 = h1; slots {0:k, 1:ks, 2:zero, 3:-q, 4:qs}
            QK = consts.tile([64, 5, L], fp8, tag="QK", name="QK")
            marenaD = consts.tile([64, n_masksd, 2, 128], fp8, tag="marenaD",
                                  name="marenaD")
            nc.sync.dma_start(wpack, wpack_d[:, :])
            nc.sync.dma_start(bias4, bias4_d[:, :])
            nc.sync.dma_start(aux, aux_d[:, :])
            for i in range(2):
                nc.sync.dma_start(xt[i][:, LC:],
                                  xT_d[128 * i:128 * (i + 1), LC:])


            # DoubleRow PV stationaries (j-stride 144, 16B-aligned); both
            # span 128 cols so each chain covers all 128 psum rows at PE
            # column position 0 (DR matmuls reject nonzero col positions).
            # v_sb1 cols [v0(0:32) | one(32) | zeros]: h0 start=True chain
            # initializes the tile (rows 64:128 <- 0).
            # v_sb2 cols [zeros(0:64) | v1(64:96) | one(96) | zeros]: h1
            # accumulates rows 64:128 on top.
            v_sb1 = consts.tile([128, NT, 144], fp8, tag="vsb1", name="v_sb1")
            v_sb2 = consts.tile([128, NT, 144], fp8, tag="vsb2", name="v_sb2")
            # reciprocal-broadcast buffers (2 rotation slots; rows 32:64
            # stay zero so the single [0:96] normalize-mul is junk-safe)
            rb_sb = consts.tile([96, 2, 512], f32, tag="rbsb", name="rb_sb")

            # PE warm-up: TRN2 pstate ramps key off continuous PE activity;
            # a few throwaway matmuls keep the projections off the cold clock.
            dummy = consts.tile([128, 512], bf16, tag="dummy", name="dummy")
            dume = consts.tile([1, 1], bf16, tag="dume", name="dume")
            nc.gpsimd.memset(dummy, 0.0)
            nc.gpsimd.memset(dume, 0.0)
            # mask arenas + the DR zero slot are zeroed during the
            # input-DMA wait (Pool is idle); marena2 first -- lc0's masks
            # gate the first exps
            nc.gpsimd.memset(marena2, 0.0)
            nc.gpsimd.memset(QK[:, 2, :], 0.0)
            nc.gpsimd.memset(marenaD, 0.0)

            # hoist the Exp act-table load off the critical path
            nc.scalar.activation(dume, dummy[0:1, 0:1],
                                 mybir.ActivationFunctionType.Exp)
            for w in range(6):
                pdum = ps_self.tile([128, 512], f32, tag="self",
                                    name="ps_self_t")
                nc.tensor.matmul(pdum, dummy[:, 0:128], dummy,
                                 start=True, stop=True)


            def proj(lc):
                sl = slice(LC * lc, LC * (lc + 1))
                pk = ps_self.tile([128, 512], f32, tag="self", name="ps_self_t")
                nc.tensor.matmul(pk, wk[0], xs(0, sl.start, sl.stop),
                                 start=True, stop=False)
                nc.tensor.matmul(pk, wk[1], xs(1, sl.start, sl.stop),
                                 start=False, stop=True)
                nc.vector.tensor_scalar(
                    out=kstg[lc], in0=pk, scalar1=bk_t, scalar2=None, op0=add)
                # fold K into the DR pack (partitions 0:32 h0, 32:64 h1)
                nc.sync.dma_start(QK[0:32, 0, sl], kstg[lc][0:32, :])
                nc.sync.dma_start(QK[0:32, 1, sl], kstg[lc][32:64, :])
                nc.sync.dma_start(QK[32:64, 0, sl], kstg[lc][64:96, :])
                nc.sync.dma_start(QK[32:64, 1, sl], kstg[lc][96:128, :])
                pq = ps_self.tile([128, 512], f32, tag="self", name="ps_self_t")
                nc.tensor.matmul(pq, wq[0], xs(0, sl.start, sl.stop),
                                 start=True, stop=False)
                nc.tensor.matmul(pq, wq[1], xs(1, sl.start, sl.stop),
                                 start=False, stop=True)
                nc.vector.tensor_scalar(
                    out=qstg[lc], in0=pq, scalar1=sg_t, scalar2=bqs_t,
                    op0=mult, op1=add)
                if lc != 0:
                    nc.sync.dma_start(QK[0:32, 3, sl], qstg[lc][0:32, :])
                    nc.sync.dma_start(QK[0:32, 4, sl], qstg[lc][32:64, :])
                    nc.sync.dma_start(QK[32:64, 3, sl], qstg[lc][64:96, :])
                    nc.sync.dma_start(QK[32:64, 4, sl], qstg[lc][96:128, :])
                # sign-baked masks built straight off this chunk's staging
                # on the (idle) Pool engine; DR-layout masks come from the
                # folded pack (after the K-fold DMAs land)
                for (t, r0, r1), m in sorted(mask0_idx.items(),
                                             key=lambda kv: kv[1]):
                    if t // 4 == lc:
                        tl = 128 * (t % 4)
                        nc.gpsimd.tensor_scalar(
                            out=marena2[:, m, r0:r1],
                            in0=kstg[lc][:, tl + r0:tl + r1],
                            scalar1=-1.0, scalar2=None, op0=mult)
                for (t, r0, r1), m in sorted(maskd_idx.items(),
                                             key=lambda kv: kv[1]):
                    if t // 4 == lc:
                        nc.gpsimd.tensor_scalar(
                            out=marenaD[:, m, :, r0:r1],
                            in0=QK[:, 0:2, 128 * t + r0:128 * t + r1],
                            scalar1=-1.0, scalar2=None, op0=mult)

            def vbuild():
                # PV-stationary + rb zero-fills sit here in Pool program
                # order: after the lc0/lc1 mask copies (which gate the first
                # exps), well before PV/normalize need them
                nc.gpsimd.memset(v_sb1, 0.0)
                nc.gpsimd.memset(v_sb1[:, :, 32:33], 1.0)
                nc.gpsimd.memset(v_sb2, 0.0)
                nc.gpsimd.memset(v_sb2[:, :, 96:97], 1.0)
                nc.gpsimd.memset(rb_sb[32:64, :, :], 0.0)
                # 4 t-tiles per psum tile; strided 4-block copies into the
                # two PV stationaries
                for q in range(NT // 4):
                    pv = ps_self.tile([128, 512], f32, tag="self",
                                      name="ps_self_t")
                    for k in range(4):
                        t = 4 * q + k
                        ts = slice(128 * t, 128 * (t + 1))
                        nc.tensor.matmul(pv[:, 128 * k:128 * k + 64],
                                         xs(0, ts.start, ts.stop),
                                         wv[0], start=True, stop=False)
                        nc.tensor.matmul(pv[:, 128 * k:128 * k + 64],
                                         xs(1, ts.start, ts.stop),
                                         wv[1], start=False, stop=True)
                    src = pv.rearrange("p (t u) -> p t u", t=4)
                    nc.vector.tensor_copy(
                        v_sb1[:, 4 * q:4 * q + 4, 0:32], src[:, :, 0:32])
                    nc.vector.tensor_copy(
                        v_sb2[:, 4 * q:4 * q + 4, 64:96], src[:, :, 32:64])

            pslabs = {}
            oaccs = {}
            ons = {}

            # score groups: pairs of t-tiles -> 1024-wide exps
            SGROUPS = [[0, 1], [2, 3], [4, 5], [6, 7], [8, 9], [10, 11],
                       [12, 13], [14, 15]]
            # unit 0 leads with narrow groups: exp0 fires right after the
            # first mains land instead of waiting for a full 1024-wide group
            SGROUPS0 = [[0], [1], [2, 3], [4, 5], [6, 7], [8, 9], [10, 11],
                        [12, 13], [14, 15]]
            DVE_EXP_GROUPS = {0, 2, 4}
            U0_DVE = 1
            NSELF = 2
            DVE_EXP_GROUPS_ODD = {0, 2, 4}
            DVE_EXP_GROUPS_LAST = {1, 3, 5}
            SCH_SCALE = float(-8.0 / np.log(2.0))   # negated Schraudolph
            SCH_BIAS = 56.0 - 0.45

            def S(u, g0=0, g1=None):
                lc, h = u // 2, u % 2
                groups = SGROUPS0 if u == 0 else SGROUPS
                if g1 is None:
                    g1 = len(groups)
                l0 = LC * lc
                if g0 == 0:
                    pslab = pslab_pool.tile([128, NT * 512], fp8, tag="pslab",
                                            name="pslab")
                    pslabs[u] = pslab
                else:
                    pslab = pslabs[u]

                def score_tiles_into(ps2, ts):
                    for k, t in enumerate(ts):
                        o = 512 * k
                        rl = rects.get((t, lc), [])
                        if lc == 0:
                            # staging-direct path: plain fp8 matmuls straight
                            # from kstg/qstg -- no fold-DMA round-trip on the
                            # pipeline-fill critical path
                            kc, tl = kstg[t // 4], 128 * (t % 4)
                            hs = slice(64 * h, 64 * h + 32)
                            hd = slice(64 * h, 64 * h + 64)
                            nc.tensor.matmul(
                                ps2[:, o:o + 512],
                                kc[hs, tl:tl + 128], qstg[0][hs, :],
                                start=True, stop=(not rl),
                                tile_position=(64 * h, 0))
                            for i, (r0, r1, c0, c1) in enumerate(rl):
                                m = mask0_idx[(t, r0, r1)]
                                nc.tensor.matmul(
                                    ps2[:, o + c0:o + c1],
                                    marena2[hd, m, :], qstg[0][hd, c0:c1],
                                    start=False, stop=(i == len(rl) - 1),
                                    tile_position=(64 * h, 0))
                            continue
                        # DoubleRow path (lc>0): mains pair [k | 0] via the
                        # stride-2 slot view; deltas pair -[k~ | k~s]
                        hp = slice(32 * h, 32 * h + 32)
                        nc.tensor.matmul(
                            ps2[:, o:o + 512],
                            QK[hp, 0:3:2, 128 * t:128 * (t + 1)],
                            QK[hp, 3:5, LC * lc:LC * (lc + 1)],
                            start=True, stop=(not rl),
                            perf_mode=DR, tile_position=(32 * h, 0))
                        for i, (r0, r1, c0, c1) in enumerate(rl):
                            m = maskd_idx[(t, r0, r1)]
                            nc.tensor.matmul(
                                ps2[:, o + c0:o + c1],
                                marenaD[hp, m, :, :],
                                QK[hp, 3:5, LC * lc + c0:LC * lc + c1],
                                start=False, stop=(i == len(rl) - 1),
                                perf_mode=DR, tile_position=(32 * h, 0))

                for g in range(g0, g1):
                    ts = groups[g]
                    w = 512 * len(ts)
                    if g >= len(groups) - NSELF and len(ts) == 2:
                        # last groups run from the (mostly idle) self pool as
                        # two 512-wide tiles: the NEXT unit's first group then
                        # reuses a score buffer freed two groups earlier
                        for t in ts:
                            pa = ps_self.tile([128, 512], f32, tag="self",
                                              name="ps_self_t")
                            score_tiles_into(pa, [t])
                            nc.scalar.activation(
                                pslab[:, 512 * t:512 * (t + 1)], pa,
                                mybir.ActivationFunctionType.Exp, scale=-1.0)
                        continue
                    ps2 = ps_score.tile([128, 1024], f32, tag="score",
                                        name="ps2")
                    score_tiles_into(ps2, ts)
                    dst = pslab[:, 512 * ts[0]:512 * ts[0] + w]
                    if u == NLC * 2 - 1:
                        dve_g = DVE_EXP_GROUPS_LAST
                    elif u % 2 == 1:
                        dve_g = DVE_EXP_GROUPS_ODD
                    else:
                        dve_g = DVE_EXP_GROUPS
                    if g in dve_g and u >= U0_DVE:
                        # negated-Schraudolph exp on DVE: fp8 bits =
                        # round(-x*8/ln2 + 55.55) via int8 bitcast; softmax
                        # normalization cancels the correlated approx error
                        nc.vector.tensor_scalar(
                            out=dst.bitcast(mybir.dt.int8), in0=ps2[:, 0:w],
                            scalar1=SCH_SCALE, scalar2=SCH_BIAS,
                            op0=mult, op1=add)
                    else:
                        nc.scalar.activation(
                            dst, ps2[:, 0:w],
                            mybir.ActivationFunctionType.Exp, scale=-1.0)

            def P(u, gp0=0, gp1=None):
                # fp8 DoubleRow PV; both heads of an lc share ONE oacc tile:
                # h0 (v_sb1 cols 0:64) -> rows 0:64 (sum0 at 32); h1 (v_sb2
                # cols 0:128, zero block first) accumulates rows 64:128
                # (sum1 at 96) at PE column position 0.  Emitted in gp-chunks
                # interleaved between the next unit's score groups so PV
                # doesn't starve the exp stream in PE's in-order queue.
                lc, h = u // 2, u % 2
                if gp1 is None:
                    gp1 = NT // 2
                pslab = pslabs[u]
                if h == 0 and gp0 == 0:
                    oacc = ps_oacc.tile([128, 512], f32, tag="oacc",
                                        name="oacc")
                    oaccs[lc] = oacc
                else:
                    oacc = oaccs[lc]
                vsb = v_sb1 if h == 0 else v_sb2
                for gp in range(gp0, gp1):
                    rhs = pslab[:, 1024 * gp:1024 * (gp + 1)].rearrange(
                        "p (two f) -> p two f", two=2)
                    nc.tensor.matmul(
                        oacc, vsb[:, 2 * gp:2 * gp + 2, 0:128],
                        rhs,
                        start=(h == 0 and gp == 0),
                        stop=(h == 1 and gp == NT // 2 - 1),
                        perf_mode=DR, tile_position=(0, 0))

            def OA(lc):
                # h0 normalize, early (right after P(2lc)): recip + Pool
                # broadcast + one mul over [0:64] -- oacc rows 33:64 are 0
                # (v_sb1 zero cols) and rb rows 32:64 are 0, so the mul also
                # zeroes `on`'s middle rows for the contract-96 out-proj.
                oacc = oaccs[lc]
                i = lc % 2
                rr0 = small_pool.tile([1, 512], f32, tag="rr", name="rr")
                nc.vector.reciprocal(rr0, oacc[32:33, :])
                nc.gpsimd.partition_broadcast(rb_sb[0:32, i, :], rr0)
                on = onorm_pool.tile([96, 512], bf16, tag="onorm", name="on")
                ons[lc] = on
                nc.vector.tensor_mul(on[0:64, :], oacc[0:64, :],
                                     rb_sb[0:64, i, :])

            def OB(lc, tail=False):
                # h1 normalize + merged out-proj.  partition_broadcast only
                # writes at base 0, so h1's 1/s reaches rows 64:96 via a
                # scratch tile + DMA (off the critical path mid-flight); the
                # tail instead uses a PE f32r rank-1 matmul into psum rows
                # 64:96 and muls against an Act-made SBUF copy of the
                # numerators, skipping the DMA round-trip.
                oacc = oaccs[lc]
                i = lc % 2
                on = ons[lc]
                if not tail:
                    rr1 = small_pool.tile([1, 512], f32, tag="rr", name="rr")
                    nc.vector.reciprocal(rr1, oacc[96:97, :])
                    rbt = small_pool.tile([32, 512], f32, tag="rbt", name="rbt")
                    nc.gpsimd.partition_broadcast(rbt, rr1)
                    nc.sync.dma_start(rb_sb[64:96, i, :], rbt)
                    nc.vector.tensor_mul(on[64:96, :], oacc[64:96, :],
                                         rb_sb[64:96, i, :])
                else:
                    rr1 = small_pool.tile([1, 512], f32r, tag="rrr", name="rrr")
                    with nc.allow_low_precision(reason="f32r recip for PE bcast"):
                        nc.vector.reciprocal(rr1, oacc[96:97, :])
                    nb = small_pool.tile([32, 512], f32, tag="nb", name="nb")
                    nc.scalar.copy(nb, oacc[64:96, :])
                    rbp = ps_self.tile([128, 512], f32, tag="self",
                                       name="ps_self_t")
                    nc.tensor.matmul(rbp, aux, rr1, start=True, stop=True)
                    nc.vector.tensor_mul(on[64:96, :], nb, rbp[64:96, :])
                for jj in range(2):
                    po = ps_self.tile([128, 512], f32, tag="self",
                                      name="ps_self_t")
                    for j2 in range(2):
                        j = 2 * jj + j2
                        onj = on[:, 128 * j:128 * (j + 1)]
                        nc.tensor.matmul(po[:, 256 * j2:256 * (j2 + 1)],
                                         onj, wo96, start=True, stop=True)
                    osb = outsb_pool.tile([128, 512], bf16, tag="outsb",
                                          name="osb")
                    if tail and jj == 0:
                        nc.scalar.activation(osb, po,
                                             mybir.ActivationFunctionType.Copy)
                    else:
                        nc.vector.tensor_copy(osb, po)
                    r0 = LC * lc + 256 * jj
                    dst = out_d[r0:r0 + 256, :].rearrange(
                        "(j r) c -> r j c", j=2)
                    nc.sync.dma_start(dst, osb.rearrange("r (j c) -> r j c", j=2))

            # ---- software-pipelined emission ------------------------------
            # S(0)'s group g reads t-tiles of lc chunk ~g//2, so interleaving
            # with proj(lc) is program-order safe
            proj(0)
            S(0, 0, 3)
            proj(1)
            S(0, 3, 5)
            proj(2)
            S(0, 5, 7)
            proj(3)
            S(0, 7, 9)
            S(1)
            vbuild()
            S(2, 0, 2)
            P(0, 0, 4)
            S(2, 2, 6)
            P(0, 4, 8)
            S(2, 6, 8)
            S(3, 0, 2)
            P(1, 0, 4)
            S(3, 2, 6)
            P(1, 4, 8)
            S(3, 6, 8)
            OA(0)
            OB(0)
            S(4, 0, 2)
            P(2, 0, 4)
            S(4, 2, 6)
            P(2, 4, 8)
            S(4, 6, 8)
            S(5, 0, 2)
            P(3, 0, 4)
            S(5, 2, 6)
            P(3, 4, 8)
            S(5, 6, 8)
            OA(1)
            OB(1)
            S(6, 0, 2)
            P(4, 0, 4)
            S(6, 2, 6)
            P(4, 4, 8)
            S(6, 6, 8)
            S(7, 0, 2)
            P(5, 0, 4)
            S(7, 2, 6)
            P(5, 4, 8)
            S(7, 6, 8)
            OA(2)
            OB(2)
            P(6)
            OA(3)
            P(7)
            OB(3, tail=True)
    nc.finalize()
    return nc


def _prep_inputs(query, in_proj_weight, in_proj_bias, in_proj_weight_self,
                 in_proj_bias_self, out_proj_weight, perm):
    """Per-core input maps (host-side transposes, permutation, scaling)."""
    scaling = np.float32(D ** -0.5)
    q_perm = np.asarray(query)[perm]          # (L, NB, E)

    Wq = np.asarray(in_proj_weight[0:E])
    Wk = np.asarray(in_proj_weight[E:2 * E])
    Wv = np.asarray(in_proj_weight[2 * E:3 * E])
    Wqs = np.asarray(in_proj_weight_self[0:E])
    Wks = np.asarray(in_proj_weight_self[E:2 * E])
    bq = np.asarray(in_proj_bias[0:E])
    bk = np.asarray(in_proj_bias[E:2 * E])
    bqs = np.asarray(in_proj_bias_self[0:E])
    bks = np.asarray(in_proj_bias_self[E:2 * E])
    WoT = np.ascontiguousarray(np.asarray(out_proj_weight).T)  # (E, E)

    xTs = [np.ascontiguousarray(q_perm[:, n, :].T).astype(bfloat16)
           for n in range(NB)]

    # per-partition sign: -1 on q rows, +1 on qs rows ([q|qs] per head)
    sgn = np.concatenate([-np.ones(32), np.ones(32),
                          -np.ones(32), np.ones(32)]).astype(np.float32)

    in_maps = []
    for c in range(NCORES):
        n = c // 4
        h0 = (2 * c) % H
        h1 = h0 + 1

        def hsl(W, h):
            return W[D * h:D * (h + 1)]

        # per-head [x | xs] pairs: mains use rows 64h:64h+32, deltas the
        # full 64-row pair -- one PE tile position per head
        wq_c = np.concatenate(
            [hsl(Wq, h0), hsl(Wqs, h0), hsl(Wq, h1), hsl(Wqs, h1)], 0) * scaling
        wk_c = np.concatenate(
            [hsl(Wk, h0), hsl(Wks, h0), hsl(Wk, h1), hsl(Wks, h1)], 0)
        wv_c = np.concatenate([hsl(Wv, h0), hsl(Wv, h1)], 0)
        bq_c = np.concatenate(
            [hsl(bq, h0), hsl(bqs, h0), hsl(bq, h1), hsl(bqs, h1)], 0) * scaling
        bk_c = np.concatenate(
            [hsl(bk, h0), hsl(bks, h0), hsl(bk, h1), hsl(bks, h1)], 0)
        wo_c = np.zeros((96, E), dtype=np.float32)
        wo_c[0:32] = WoT[D * h0:D * (h0 + 1)]
        wo_c[64:96] = WoT[D * h1:D * (h1 + 1)]

        wqkv_c = np.concatenate([wq_c.T, wk_c.T, wv_c.T], axis=1)  # (E, 320)
        bias4_c = np.stack([bk_c, sgn * bq_c, sgn,
                            np.zeros(128, np.float32)], axis=1)  # (128, 4)
        wpack = np.zeros((128, 1920), dtype=np.float32)
        wpack[:, 0:320] = wqkv_c[0:128]
        wpack[:, 320:640] = wqkv_c[128:256]
        wpack[0:96, 640:896] = wo_c                                # (96, 256)
        wpack[:, 896:1408] = xTs[n][0:128, 0:512].astype(np.float32)
        wpack[:, 1408:1920] = xTs[n][128:256, 0:512].astype(np.float32)

        aux = np.zeros((1, 128), dtype=np.float32)
        aux[0, 64:96] = 1.0
        in_maps.append({
            "xT": xTs[n],
            "wpack": wpack.astype(bfloat16),
            "bias4": np.ascontiguousarray(bias4_c).astype(np.float32),
            "aux": aux,
        })
    return in_maps


def _run(nc, in_maps, trace=False):
    from concourse.bass_utils import run_bass_kernel_spmd
    return run_bass_kernel_spmd(nc, in_maps, list(range(NCORES)), trace=trace)


def kernel(query, in_proj_weight, in_proj_bias, in_proj_weight_self,
           in_proj_bias_self, out_proj_weight, out_proj_bias,
           q_identities, k_identities, _trace=False, _return_br=False):
    ids = np.asarray(q_identities)
    perm, blocks = _block_structure(ids)

    key = ids.tobytes()
    if key not in _PROGRAM_CACHE:
        _PROGRAM_CACHE[key] = _build_program(_rects(blocks))
    nc = _PROGRAM_CACHE[key]

    in_maps = _prep_inputs(query, in_proj_weight, in_proj_bias,
                           in_proj_weight_self, in_proj_bias_self,
                           out_proj_weight, perm)
    br = _run(nc, in_maps, trace=_trace)

    # ---- unshard --------------------------------------------------------------
    # host bias: out_proj_bias + contribution of the v-bias through out_proj
    bias_total = (np.asarray(out_proj_bias)
                  + np.asarray(out_proj_weight) @ np.asarray(in_proj_bias)[2 * E:])
    out = np.zeros((L, NB, E), dtype=np.float32)
    for c in range(NCORES):
        n = c // 4
        out[:, n, :] += np.asarray(br.results[c]["out"], dtype=np.float32)
    out += bias_total[None, None, :].astype(np.float32)
    # un-permute rows
    out_full = np.empty_like(out)
    out_full[perm] = out
    if _return_br:
        return out_full, br
    return out_full
